# Initial kernel scaffold
#
"""AdaptiveMixGNNLayer Trainium2 kernel (8 NeuronCores, SPMD, no collectives).

Strategy: 1D node partition — each core owns a contiguous range of destination
rows. Host (here, inside kernel()) partitions+sorts the COO edges of both
operators by (core, 128-row destination block), pads each block to whole
128-edge tiles, and ships per-core index/value arrays. x is replicated in each
core's HBM. On device, per destination block:
  - one batched indirect DMA gathers x[col] for all of the block's edges
    (edge -> partition, tile -> free slot)
  - DVE builds a one-hot matrix P'[e, r] = val_e * (rblk_e == r) from a
    repeated iota and broadcast access patterns
  - TensorE accumulates Z[r, :] += P'^T @ Xg into PSUM (lp and hp separately)
  - epilogue: alpha-mix (per-row scale), transpose, @ W^T, +b, ReLU, DMA out.
alpha = sigmoid(x @ alpha_w^T + alpha_b) is computed on-device from the core's
own row slice.
"""
import math

import numpy as np

P = 128  # partitions / tile edge


# ---------------------------------------------------------------- host prep
def _prep_op(rows, cols, vals, n_cores, rpc, nblk):
    """Sort one operator's edges by (core, dest block); compute per-block tile
    counts (shared across cores for SPMD) and per-core padded arrays."""
    rows = np.asarray(rows)
    cols = np.asarray(cols)
    vals = np.asarray(vals)
    core = rows // rpc
    rloc = rows - core * rpc
    blk = rloc // P
    rblk = rloc - blk * P
    key = core * nblk + blk
    order = np.argsort(key, kind="stable")
    cnt = np.bincount(key, minlength=n_cores * nblk).reshape(n_cores, nblk)
    tb = np.maximum(1, -(-cnt.max(axis=0) // P)).astype(np.int64)  # [nblk]
    gstart = np.concatenate([[0], np.cumsum(cnt.reshape(-1))])
    cs, vs, rs = cols[order], vals[order], rblk[order]
    return tb, cnt, gstart, cs, vs, rs


def _pack(n_cores, nblk, tb_lp, prep_lp, tb_hp, prep_hp):
    """Interleave lp/hp per block into combined [P, TT] tile-column arrays."""
    tc = tb_lp + tb_hp
    tt = int(tc.sum())
    ct0 = np.concatenate([[0], np.cumsum(tc)])[:-1]  # tile-col base per block
    cols_a = np.zeros((n_cores, P, tt), np.int32)
    vals_a = np.zeros((n_cores, P, tt), np.float32)
    rblk_a = np.zeros((n_cores, P, tt), np.float32)

    def fill(prep, tbs, toff):
        _, cnt, gstart, cs, vs, rs = prep
        for c in range(n_cores):
            for b in range(nblk):
                g = c * nblk + b
                s, e = gstart[g], gstart[g + 1]
                n = e - s
                t0 = ct0[b] + toff[b]
                ntile = tbs[b]
                buf_c = np.zeros(ntile * P, np.int32)
                buf_v = np.zeros(ntile * P, np.float32)
                buf_r = np.zeros(ntile * P, np.float32)
                buf_c[:n] = cs[s:e]
                buf_v[:n] = vs[s:e]
                buf_r[:n] = rs[s:e]
                # edge j = t*P + p  ->  [p, t]
                cols_a[c, :, t0 : t0 + ntile] = buf_c.reshape(ntile, P).T
                vals_a[c, :, t0 : t0 + ntile] = buf_v.reshape(ntile, P).T
                rblk_a[c, :, t0 : t0 + ntile] = buf_r.reshape(ntile, P).T

    fill(prep_lp, tb_lp, np.zeros(nblk, np.int64))
    fill(prep_hp, tb_hp, tb_lp)
    return cols_a, vals_a, rblk_a, tc, ct0, tt


# ------------------------------------------------------------- bass builder
def _build(n, d, n_cores, rpc, nblk, last_rows, tb_lp, tc, ct0, tt):
    from contextlib import ExitStack

    from concourse import bacc, bass, mybir
    from concourse import tile
    from concourse.masks import make_identity

    F32 = mybir.dt.float32
    I32 = mybir.dt.int32
    Relu = mybir.ActivationFunctionType.Relu
    Sigmoid = mybir.ActivationFunctionType.Sigmoid
    Copy = mybir.ActivationFunctionType.Copy
    Alu = mybir.AluOpType

    tcmax = int(tc.max())

    nc = bacc.Bacc("TRN2", target_bir_lowering=False, debug=False,
                   num_devices=n_cores)
    x_d = nc.dram_tensor("x", [n, d], F32, kind="ExternalInput")
    xo_d = nc.dram_tensor("x_own", [rpc, d], F32, kind="ExternalInput")
    wt_d = nc.dram_tensor("wt", [d, d], F32, kind="ExternalInput")
    b_d = nc.dram_tensor("bvec", [1, d], F32, kind="ExternalInput")
    aw_d = nc.dram_tensor("aw", [1, d], F32, kind="ExternalInput")
    ab_d = nc.dram_tensor("ab", [1, 1], F32, kind="ExternalInput")
    cols_d = nc.dram_tensor("cols", [P, tt], I32, kind="ExternalInput")
    vals_d = nc.dram_tensor("vals", [P, tt], F32, kind="ExternalInput")
    rblk_d = nc.dram_tensor("rblk", [P, tt], F32, kind="ExternalInput")
    out_d = nc.dram_tensor("out", [rpc, d], F32, kind="ExternalOutput")
    alpha_d = nc.dram_tensor("alpha", [rpc, 1], F32, kind="ExternalOutput")

    with ExitStack() as ctx, tile.TileContext(nc) as tc_:
        const = ctx.enter_context(tc_.tile_pool(name="const", bufs=1))
        meta = ctx.enter_context(tc_.tile_pool(name="meta", bufs=1))
        gth = ctx.enter_context(tc_.tile_pool(name="gth", bufs=3))
        pbp = ctx.enter_context(tc_.tile_pool(name="pbp", bufs=3))
        work = ctx.enter_context(tc_.tile_pool(name="work", bufs=4))
        outp = ctx.enter_context(tc_.tile_pool(name="outp", bufs=3))
        pacc = ctx.enter_context(tc_.tile_pool(name="pacc", bufs=4, space="PSUM"))
        pmisc = ctx.enter_context(tc_.tile_pool(name="pmisc", bufs=3, space="PSUM"))

        # ---- constants
        ident = const.tile([P, P], F32)
        make_identity(nc, ident[:])
        iota = const.tile([P, P * tcmax], F32)
        nc.vector.iota(iota[:], pattern=[[1, P], [0, tcmax]], base=0,
                       channel_multiplier=0,
                       allow_small_or_imprecise_dtypes=True)
        ones_col = const.tile([1, P], F32)
        nc.vector.memset(ones_col[:], 1.0)
        wt_sb = const.tile([P, d], F32)
        nc.sync.dma_start(out=wt_sb[:], in_=wt_d[:, :])
        b_sb = const.tile([1, d], F32)
        nc.sync.dma_start(out=b_sb[:], in_=b_d[:, :])
        aw_sb = const.tile([1, d], F32)
        nc.sync.dma_start(out=aw_sb[:], in_=aw_d[:, :])
        ab_sb = const.tile([1, 1], F32)
        nc.sync.dma_start(out=ab_sb[:], in_=ab_d[:, :])
        # replicate alpha_w across partitions via K=1 matmul
        ps_aw = pmisc.tile([P, d], F32, tag="ps_bc")
        nc.tensor.matmul(ps_aw[:], lhsT=ones_col[:], rhs=aw_sb[:],
                         start=True, stop=True)
        aw_rep = const.tile([P, d], F32)
        nc.vector.tensor_copy(aw_rep[:], ps_aw[:])
        ps_ab = pmisc.tile([P, 1], F32, tag="ps_bc")
        nc.tensor.matmul(ps_ab[:], lhsT=ones_col[:], rhs=ab_sb[:],
                         start=True, stop=True)
        ab_rep = const.tile([P, 1], F32)
        nc.vector.tensor_copy(ab_rep[:], ps_ab[:])

        # ---- edge metadata (whole thing resident)
        cols_sb = meta.tile([P, tt], I32)
        nc.sync.dma_start(out=cols_sb[:], in_=cols_d[:, :])
        vals_sb = meta.tile([P, tt], F32)
        nc.sync.dma_start(out=vals_sb[:], in_=vals_d[:, :])
        rblk_sb = meta.tile([P, tt], F32)
        nc.sync.dma_start(out=rblk_sb[:], in_=rblk_d[:, :])

        alpha_all = const.tile([P, nblk], F32)
        oma_all = const.tile([P, nblk], F32)

        for b in range(nblk):
            tcb = int(tc[b])
            tlp = int(tb_lp[b])
            c0 = int(ct0[b])
            nrows = last_rows if b == nblk - 1 else P

            # ---- gather all of this block's x rows (lp then hp tiles)
            xg = gth.tile([P, tcmax * d], F32, tag="xg")
            nc.gpsimd.indirect_dma_start(
                out=xg[:, : tcb * d],
                out_offset=None,
                in_=x_d[:, :],
                in_offset=bass.IndirectOffsetOnAxis(
                    ap=cols_sb[:, c0 : c0 + tcb], axis=0),
            )

            # ---- one-hot P'[e, (j, t)] = (j == rblk[e,t]) * val[e,t]
            pb = pbp.tile([P, P * tcmax], F32, tag="pb")
            pb3 = pb[:].rearrange("p (j t) -> p j t", j=P, t=tcmax)[:, :, :tcb]
            iota3 = iota[:].rearrange("p (j t) -> p j t", j=P, t=tcmax)[:, :, :tcb]
            rb_b = rblk_sb[:, c0 : c0 + tcb].unsqueeze(1).to_broadcast(
                [P, P, tcb])
            va_b = vals_sb[:, c0 : c0 + tcb].unsqueeze(1).to_broadcast(
                [P, P, tcb])
            nc.vector.tensor_tensor(out=pb3, in0=iota3, in1=rb_b,
                                    op=Alu.is_equal)
            nc.vector.tensor_tensor(out=pb3, in0=pb3, in1=va_b, op=Alu.mult)

            # ---- accumulate Z_lp, Z_hp in PSUM: Z[r, f] += P'^T @ Xg
            ps_lp = pacc.tile([P, d], F32, tag="ps_lp")
            ps_hp = pacc.tile([P, d], F32, tag="ps_hp")
            for t in range(tcb):
                is_lp = t < tlp
                ps = ps_lp if is_lp else ps_hp
                nc.tensor.matmul(
                    ps[:],
                    lhsT=pb3[:, :, t],
                    rhs=xg[:, t * d : (t + 1) * d],
                    start=(t == 0 or t == tlp),
                    stop=(t == tlp - 1 or t == tcb - 1),
                )

            # ---- alpha for this block's own rows
            xo_t = work.tile([P, d], F32, tag="xo")
            if nrows < P:
                nc.vector.memset(xo_t[:], 0.0)
            nc.sync.dma_start(out=xo_t[:nrows, :],
                              in_=xo_d[b * P : b * P + nrows, :])
            ttr = work.tile([P, d], F32, tag="ttr")
            nc.vector.tensor_tensor_reduce(
                out=ttr[:], in0=xo_t[:], in1=aw_rep[:], scale=1.0, scalar=0.0,
                op0=Alu.mult, op1=Alu.add,
                accum_out=alpha_all[:, b : b + 1])
            nc.scalar.activation(alpha_all[:, b : b + 1],
                                 alpha_all[:, b : b + 1],
                                 Sigmoid, bias=ab_rep[:], scale=1.0)
            nc.vector.tensor_scalar(out=oma_all[:, b : b + 1],
                                    in0=alpha_all[:, b : b + 1],
                                    scalar1=-1.0, scalar2=1.0,
                                    op0=Alu.mult, op1=Alu.add)

            # ---- mix: z = alpha * z_lp + (1 - alpha) * z_hp  (per-row scale)
            mx_lp = work.tile([P, d], F32, tag="mx_lp")
            nc.scalar.activation(mx_lp[:], ps_lp[:], Copy,
                                 scale=alpha_all[:, b : b + 1])
            mx_hp = work.tile([P, d], F32, tag="mx_hp")
            nc.scalar.activation(mx_hp[:], ps_hp[:], Copy,
                                 scale=oma_all[:, b : b + 1])
            zmix = work.tile([P, d], F32, tag="zmix")
            nc.vector.tensor_tensor(out=zmix[:], in0=mx_lp[:], in1=mx_hp[:],
                                    op=Alu.add)

            # ---- out = relu(zmix @ W^T + b): transpose zmix, then matmul
            ps_t = pmisc.tile([P, P], F32, tag="ps_t")
            nc.tensor.transpose(ps_t[:], zmix[:], ident[:])
            zt = work.tile([P, P], F32, tag="zt")
            nc.vector.tensor_copy(zt[:], ps_t[:])
            ps_o = pmisc.tile([P, d], F32, tag="ps_o")
            nc.tensor.matmul(ps_o[:], lhsT=zt[:], rhs=wt_sb[:],
                             start=True, stop=False)
            nc.tensor.matmul(ps_o[:], lhsT=ones_col[:], rhs=b_sb[:],
                             start=False, stop=True)
            o_sb = outp.tile([P, d], F32, tag="o_sb")
            nc.scalar.activation(o_sb[:], ps_o[:], Relu)
            nc.sync.dma_start(out=out_d[b * P : b * P + nrows, :],
                              in_=o_sb[:nrows, :])

        # ---- alpha output: transpose [P, nblk] -> [nblk, P] and store
        ps_at = pmisc.tile([P, P], F32, tag="ps_t")
        nc.tensor.transpose(ps_at[:, :], alpha_all[:].to_broadcast([P, nblk]),
                            ident[:]) if False else None
        # transpose wants in_ [P, nblk] -> out [nblk, P]
        nc.tensor.transpose(ps_at[:nblk, :], alpha_all[:], ident[:])
        at_sb = outp.tile([P, P], F32, tag="at_sb")
        nc.vector.tensor_copy(at_sb[:nblk, :], ps_at[:nblk, :])
        nfull = nblk - 1
        if nfull > 0:
            nc.sync.dma_start(
                out=alpha_d[: nfull * P, 0].rearrange("(b r) -> b r", r=P),
                in_=at_sb[:nfull, :])
        nc.sync.dma_start(out=alpha_d[nfull * P : nfull * P + last_rows, 0]
                          .rearrange("(b r) -> b r", r=last_rows),
                          in_=at_sb[nfull : nfull + 1, :last_rows])

    nc.compile()
    return nc


# ------------------------------------------------------------------ driver
def _run(inputs, n_cores=8, trace=False):
    from concourse.bass_utils import run_bass_kernel_spmd

    x = np.asarray(inputs["x"], np.float32)
    n, d = x.shape
    rpc = n // n_cores
    nblk = -(-rpc // P)
    last_rows = rpc - (nblk - 1) * P

    prep_lp = _prep_op(inputs["lp_rows"], inputs["lp_cols"], inputs["lp_vals"],
                       n_cores, rpc, nblk)
    prep_hp = _prep_op(inputs["hp_rows"], inputs["hp_cols"], inputs["hp_vals"],
                       n_cores, rpc, nblk)
    tb_lp, tb_hp = prep_lp[0], prep_hp[0]
    cols_a, vals_a, rblk_a, tc, ct0, tt = _pack(
        n_cores, nblk, tb_lp, prep_lp, tb_hp, prep_hp)

    nc = _build(n, d, n_cores, rpc, nblk, last_rows, tb_lp, tc, ct0, tt)

    wt = np.ascontiguousarray(np.asarray(inputs["W"], np.float32).T)
    bvec = np.asarray(inputs["b"], np.float32).reshape(1, d)
    aw = np.asarray(inputs["alpha_w"], np.float32).reshape(1, d)
    ab = np.asarray(inputs["alpha_b"], np.float32).reshape(1, 1)
    in_maps = []
    for c in range(n_cores):
        in_maps.append({
            "x": x,
            "x_own": np.ascontiguousarray(x[c * rpc : (c + 1) * rpc]),
            "wt": wt, "bvec": bvec, "aw": aw, "ab": ab,
            "cols": np.ascontiguousarray(cols_a[c]),
            "vals": np.ascontiguousarray(vals_a[c]),
            "rblk": np.ascontiguousarray(rblk_a[c]),
        })
    res = run_bass_kernel_spmd(nc, in_maps, core_ids=list(range(n_cores)),
                               trace=trace)
    out = np.concatenate([res.results[c]["out"] for c in range(n_cores)], 0)
    alpha = np.concatenate([res.results[c]["alpha"] for c in range(n_cores)], 0)
    return (out, alpha), res


def kernel(**inputs):
    (out, alpha), _ = _run(inputs, trace=False)
    return out, alpha


# revision 10
# speedup vs baseline: 30.3607x; 30.3607x over previous
"""AdaptiveMixGNNLayer Trainium2 kernel (8 NeuronCores, SPMD, no collectives).

Strategy: 1D node partition — each core owns a contiguous range of destination
rows (rpc = N/8). Host (inside kernel()) partitions+sorts the COO edges of both
operators by (core, 128-row destination block, x-half), pads each segment to
whole 128-edge tiles, and ships per-core int16 gather indices + bf16 edge
values. x is replicated in each core's HBM as two bf16 halves (dma_gather
indices are int16, so the gather table must stay under 32768 rows).

Per destination block on device:
  - dma_gather pulls x[col] for all the block's edges from the low half, then
    the high half (edge -> partition i%128, tile -> free slot i//128)
  - DVE builds one-hot P'[e, (j, t)] = val * (rblk == j) from a repeated iota
    (j-major so every operand's last AP dim is packed) via broadcast APs
  - TensorE accumulates Z[r, :] += P'_t^T @ Xg_t into PSUM (lp / hp separate)
  - epilogue: alpha-mix (per-row scale on ACT), TensorE transpose, @ W^T, +b,
    ReLU, DMA out.
alpha = sigmoid(x @ alpha_w^T + alpha_b) is computed on-device in f32 from the
core's own row slice.
"""
import numpy as np

P = 128  # partitions / tile edge


# ---------------------------------------------------------------- host prep
def _prep_op(rows, cols, vals, n_cores, rpc, nblk, half):
    """Sort one operator's edges by (core, dest block, col half).

    Returns (tlo, thi) per-block tile counts (max over cores, SPMD-shared) and
    per-(core, block, half) segment slices of the sorted arrays.
    """
    rows = np.asarray(rows)
    cols = np.asarray(cols)
    vals = np.asarray(vals)
    core = rows // rpc
    rloc = rows - core * rpc
    blk = rloc // P
    rblk = (rloc - blk * P).astype(np.float32)
    hi = (cols >= half).astype(np.int64)
    key = (core * nblk + blk) * 2 + hi
    order = np.argsort(key, kind="stable")
    cnt = np.bincount(key, minlength=n_cores * nblk * 2).reshape(
        n_cores, nblk, 2)
    tlo = -(-cnt[:, :, 0].max(axis=0) // P)
    thi = -(-cnt[:, :, 1].max(axis=0) // P)
    # every block needs >= 1 tile for this operator so its PSUM group exists
    empty = (tlo + thi) == 0
    tlo[empty] = 1
    gstart = np.concatenate([[0], np.cumsum(cnt.reshape(-1))])
    cs = cols[order].astype(np.int32)
    cs[hi[order] == 1] -= half
    vs = vals[order].astype(np.float32)
    rs = rblk[order]
    return tlo.astype(np.int64), thi.astype(np.int64), gstart, cs, vs, rs


def _pack(n_cores, nblk, prep_lp, prep_hp):
    """Per-block tile order [lp_lo | hp_lo | lp_hi | hp_hi]; build combined
    per-core idx16 / vals / rblk arrays in tile-column layout."""
    import ml_dtypes

    lp_lo, lp_hi = prep_lp[0], prep_lp[1]
    hp_lo, hp_hi = prep_hp[0], prep_hp[1]
    segs = np.stack([lp_lo, hp_lo, lp_hi, hp_hi], axis=1)  # [nblk, 4]
    tc = segs.sum(axis=1)
    tt = int(tc.sum())
    ct0 = np.concatenate([[0], np.cumsum(tc)])[:-1]
    idx_a = np.zeros((n_cores, P, tt * 8), np.int16)
    vals_a = np.zeros((n_cores, P, tt), ml_dtypes.bfloat16)
    rblk_a = np.zeros((n_cores, P, tt), ml_dtypes.bfloat16)

    preps = {0: prep_lp, 1: prep_hp}
    for c in range(n_cores):
        for b in range(nblk):
            t0 = int(ct0[b])
            # segment order: (op, half): (0,0), (1,0), (0,1), (1,1)
            seg_list = [(0, 0), (1, 0), (0, 1), (1, 1)]
            tile_off = 0
            gather_cols = {0: [], 1: []}  # half -> list of int16 idx arrays
            for si, (op, hf) in enumerate(seg_list):
                ntile = int(segs[b, si])
                if ntile == 0:
                    continue
                _, _, gstart, cs, vs, rs = preps[op]
                g = (c * nblk + b) * 2 + hf
                s, e = gstart[g], gstart[g + 1]
                npad = ntile * P
                n_real = min(int(e - s), npad)
                bc = np.zeros(npad, np.int32)
                bv = np.zeros(npad, np.float32)
                br = np.zeros(npad, np.float32)
                bc[:n_real] = cs[s : s + n_real]
                bv[:n_real] = vs[s : s + n_real]
                br[:n_real] = rs[s : s + n_real]
                tcol = t0 + tile_off
                vals_a[c, :, tcol : tcol + ntile] = (
                    bv.reshape(ntile, P).T.astype(ml_dtypes.bfloat16))
                rblk_a[c, :, tcol : tcol + ntile] = (
                    br.reshape(ntile, P).T.astype(ml_dtypes.bfloat16))
                gather_cols[hf].append(bc.astype(np.int16))
                tile_off += ntile
            # idx16 layout per gather: flat i at [i%16, i//16], replicated x8
            gpos = t0
            for hf in (0, 1):
                if not gather_cols[hf]:
                    continue
                flat = np.concatenate(gather_cols[hf])
                ntile = len(flat) // P
                arr = flat.reshape(-1, 16).T  # [16, ntile*8]
                idx_a[c, :, gpos * 8 : (gpos + ntile) * 8] = np.tile(arr, (8, 1))
                gpos += ntile
    return idx_a, vals_a, rblk_a, segs, tc, ct0, tt


# ------------------------------------------------------------- bass builder
def _build(n, d, n_cores, rpc, nblk, last_rows, segs, tc, ct0, tt, half):
    from contextlib import ExitStack

    from concourse import bacc, mybir
    from concourse import tile
    from concourse.masks import make_identity

    F32 = mybir.dt.float32
    BF16 = mybir.dt.bfloat16
    I16 = mybir.dt.int16
    Relu = mybir.ActivationFunctionType.Relu
    Sigmoid = mybir.ActivationFunctionType.Sigmoid
    Copy = mybir.ActivationFunctionType.Copy
    Alu = mybir.AluOpType

    tcmax = int(tc.max())

    nc = bacc.Bacc("TRN2", target_bir_lowering=False, debug=False,
                   num_devices=n_cores)
    x0_d = nc.dram_tensor("x0b", [half, d], BF16, kind="ExternalInput")
    x1_d = nc.dram_tensor("x1b", [n - half, d], BF16, kind="ExternalInput")
    xo_d = nc.dram_tensor("x_own", [rpc, d], F32, kind="ExternalInput")
    wt_d = nc.dram_tensor("wt", [d, d], F32, kind="ExternalInput")
    b_d = nc.dram_tensor("bvec", [1, d], F32, kind="ExternalInput")
    aw_d = nc.dram_tensor("aw", [1, d], F32, kind="ExternalInput")
    ab_d = nc.dram_tensor("ab", [1, 1], F32, kind="ExternalInput")
    idx_d = nc.dram_tensor("idx", [P, tt * 8], I16, kind="ExternalInput")
    vals_d = nc.dram_tensor("vals", [P, tt], BF16, kind="ExternalInput")
    rblk_d = nc.dram_tensor("rblk", [P, tt], BF16, kind="ExternalInput")
    out_d = nc.dram_tensor("out", [rpc, d], F32, kind="ExternalOutput")
    alpha_d = nc.dram_tensor("alpha", [rpc, 1], F32, kind="ExternalOutput")

    with tile.TileContext(nc) as tc_, ExitStack() as ctx:
        const = ctx.enter_context(tc_.tile_pool(name="const", bufs=1))
        meta = ctx.enter_context(tc_.tile_pool(name="meta", bufs=1))
        gth = ctx.enter_context(tc_.tile_pool(name="gth", bufs=3))
        pbp = ctx.enter_context(tc_.tile_pool(name="pbp", bufs=3))
        work = ctx.enter_context(tc_.tile_pool(name="work", bufs=4))
        outp = ctx.enter_context(tc_.tile_pool(name="outp", bufs=3))
        pacc = ctx.enter_context(tc_.tile_pool(name="pacc", bufs=2, space="PSUM"))
        pmisc = ctx.enter_context(tc_.tile_pool(name="pmisc", bufs=2, space="PSUM"))

        # ---- constants
        ident = const.tile([P, P], F32)
        make_identity(nc, ident[:])
        iota = const.tile([P, P * tcmax], BF16)
        nc.gpsimd.iota(iota[:], pattern=[[1, P], [0, tcmax]], base=0,
                       channel_multiplier=0,
                       allow_small_or_imprecise_dtypes=True)
        ones_col = const.tile([1, P], F32)
        nc.vector.memset(ones_col[:], 1.0)
        wt_sb = const.tile([P, d], F32)
        nc.sync.dma_start(out=wt_sb[:], in_=wt_d[:, :])
        b_sb = const.tile([1, d], F32)
        nc.sync.dma_start(out=b_sb[:], in_=b_d[:, :])
        aw_sb = const.tile([1, d], F32)
        nc.sync.dma_start(out=aw_sb[:], in_=aw_d[:, :])
        ab_sb = const.tile([1, 1], F32)
        nc.sync.dma_start(out=ab_sb[:], in_=ab_d[:, :])
        # replicate alpha_w / alpha_b across partitions via K=1 matmul
        ps_aw = pmisc.tile([P, d], F32, tag="ps_t")
        nc.tensor.matmul(ps_aw[:], lhsT=ones_col[:], rhs=aw_sb[:],
                         start=True, stop=True)
        aw_rep = const.tile([P, d], F32)
        nc.vector.tensor_copy(aw_rep[:], ps_aw[:])
        ps_ab = pmisc.tile([P, 1], F32, tag="ps_t")
        nc.tensor.matmul(ps_ab[:], lhsT=ones_col[:], rhs=ab_sb[:],
                         start=True, stop=True)
        ab_rep = const.tile([P, 1], F32)
        nc.vector.tensor_copy(ab_rep[:], ps_ab[:])

        # ---- edge metadata (whole thing resident)
        idx_sb = meta.tile([P, tt * 8], I16)
        nc.sync.dma_start(out=idx_sb[:], in_=idx_d[:, :])
        vals_sb = meta.tile([P, tt], BF16)
        nc.sync.dma_start(out=vals_sb[:], in_=vals_d[:, :])
        rblk_sb = meta.tile([P, tt], BF16)
        nc.sync.dma_start(out=rblk_sb[:], in_=rblk_d[:, :])

        alpha_all = const.tile([P, nblk], F32)
        oma_all = const.tile([P, nblk], F32)

        for b in range(nblk):
            tcb = int(tc[b])
            s0, s1, s2, s3 = (int(v) for v in segs[b])
            tl, th = s0 + s1, s2 + s3
            c0 = int(ct0[b])
            nrows = last_rows if b == nblk - 1 else P

            # ---- gather the block's x rows: low half then high half
            xg = gth.tile([P, tcmax * d], BF16, tag="xg")
            if tl > 0:
                nc.gpsimd.dma_gather(
                    out_ap=xg[:, : tl * d].rearrange("p (t f) -> p t f", f=d),
                    in_ap=x0_d[:, :],
                    idxs_ap=idx_sb[:, c0 * 8 : (c0 + tl) * 8],
                    num_idxs=tl * P, num_idxs_reg=tl * P, elem_size=d)
            if th > 0:
                nc.gpsimd.dma_gather(
                    out_ap=xg[:, tl * d : tcb * d].rearrange(
                        "p (t f) -> p t f", f=d),
                    in_ap=x1_d[:, :],
                    idxs_ap=idx_sb[:, (c0 + tl) * 8 : (c0 + tcb) * 8],
                    num_idxs=th * P, num_idxs_reg=th * P, elem_size=d)

            # ---- one-hot P'[e, (j, t)] = (j == rblk[e,t]) * val[e,t]
            pb = pbp.tile([P, P * tcmax], BF16, tag="pb")
            pb3 = pb[:].rearrange("p (j t) -> p j t", j=P, t=tcmax)[:, :, :tcb]
            iota3 = iota[:].rearrange("p (j t) -> p j t", j=P, t=tcmax)[:, :, :tcb]
            rb_b = rblk_sb[:, c0 : c0 + tcb].unsqueeze(1).to_broadcast(
                [P, P, tcb])
            va_b = vals_sb[:, c0 : c0 + tcb].unsqueeze(1).to_broadcast(
                [P, P, tcb])
            nc.vector.tensor_tensor(out=pb3, in0=iota3, in1=rb_b,
                                    op=Alu.is_equal)
            nc.vector.tensor_tensor(out=pb3, in0=pb3, in1=va_b, op=Alu.mult)

            # ---- accumulate Z_lp, Z_hp in PSUM: Z[r, f] += P'_t^T @ Xg_t
            # tile roles: [0,s0) lp | [s0,tl) hp | [tl,tl+s2) lp | rest hp
            lp_tiles = list(range(0, s0)) + list(range(tl, tl + s2))
            hp_tiles = list(range(s0, tl)) + list(range(tl + s2, tcb))
            ps_lp = pacc.tile([P, d], F32, tag="ps_lp")
            ps_hp = pacc.tile([P, d], F32, tag="ps_hp")
            for t in range(tcb):
                is_lp = t in lp_tiles
                group = lp_tiles if is_lp else hp_tiles
                ps = ps_lp if is_lp else ps_hp
                nc.tensor.matmul(
                    ps[:],
                    lhsT=pb3[:, :, t],
                    rhs=xg[:, t * d : (t + 1) * d],
                    start=(t == group[0]),
                    stop=(t == group[-1]),
                )

            # ---- alpha for this block's own rows (f32)
            xo_t = work.tile([P, d], F32, tag="xo")
            if nrows < P:
                nc.vector.memset(xo_t[:], 0.0)
            nc.sync.dma_start(out=xo_t[:nrows, :],
                              in_=xo_d[b * P : b * P + nrows, :])
            ttr = work.tile([P, d], F32, tag="ttr")
            ttr2 = work.tile([P, d], F32, tag="ttr2")
            apre = work.tile([P, 1], F32, tag="apre")
            nc.vector.tensor_tensor(out=ttr[:], in0=xo_t[:], in1=aw_rep[:],
                                    op=Alu.mult)
            nc.scalar.activation(ttr2[:], ttr[:], Copy, accum_out=apre[:])
            nc.scalar.activation(alpha_all[:, b : b + 1], apre[:],
                                 Sigmoid, bias=ab_rep[:], scale=1.0)
            nc.vector.tensor_scalar(out=oma_all[:, b : b + 1],
                                    in0=alpha_all[:, b : b + 1],
                                    scalar1=-1.0, scalar2=1.0,
                                    op0=Alu.mult, op1=Alu.add)

            # ---- mix: z = alpha * z_lp + (1 - alpha) * z_hp  (per-row scale)
            mx_lp = work.tile([P, d], F32, tag="mx_lp")
            nc.scalar.activation(mx_lp[:], ps_lp[:], Copy,
                                 scale=alpha_all[:, b : b + 1])
            mx_hp = work.tile([P, d], F32, tag="mx_hp")
            nc.scalar.activation(mx_hp[:], ps_hp[:], Copy,
                                 scale=oma_all[:, b : b + 1])
            zmix = work.tile([P, d], F32, tag="zmix")
            nc.vector.tensor_tensor(out=zmix[:], in0=mx_lp[:], in1=mx_hp[:],
                                    op=Alu.add)

            # ---- out = relu(zmix @ W^T + b): transpose zmix, then matmul
            ps_t = pmisc.tile([P, P], F32, tag="ps_t")
            nc.tensor.transpose(ps_t[:], zmix[:], ident[:])
            zt = work.tile([P, P], F32, tag="zt")
            nc.vector.tensor_copy(zt[:], ps_t[:])
            ps_o = pmisc.tile([P, d], F32, tag="ps_o")
            nc.tensor.matmul(ps_o[:], lhsT=zt[:], rhs=wt_sb[:],
                             start=True, stop=False)
            nc.tensor.matmul(ps_o[:], lhsT=ones_col[:], rhs=b_sb[:],
                             start=False, stop=True)
            o_sb = outp.tile([P, d], F32, tag="o_sb")
            nc.scalar.activation(o_sb[:], ps_o[:], Relu)
            nc.sync.dma_start(out=out_d[b * P : b * P + nrows, :],
                              in_=o_sb[:nrows, :])

        # ---- alpha output: transpose [P, nblk] -> [nblk, P] and store
        ps_at = pmisc.tile([P, P], F32, tag="ps_t")
        nc.tensor.transpose(ps_at[:nblk, :], alpha_all[:], ident[:])
        at_sb = outp.tile([P, P], F32, tag="at_sb")
        nc.vector.tensor_copy(at_sb[:nblk, :], ps_at[:nblk, :])
        nfull = nblk - 1
        if nfull > 0:
            nc.sync.dma_start(
                out=alpha_d[: nfull * P, 0].rearrange("(b r) -> b r", r=P),
                in_=at_sb[:nfull, :])
        nc.sync.dma_start(out=alpha_d[nfull * P : nfull * P + last_rows, 0]
                          .rearrange("(b r) -> b r", r=last_rows),
                          in_=at_sb[nfull : nfull + 1, :last_rows])

    nc.compile()
    return nc


# ------------------------------------------------------------------ driver
def _make(inputs, n_cores=8):
    import ml_dtypes

    x = np.asarray(inputs["x"], np.float32)
    n, d = x.shape
    half = n // 2
    rpc = n // n_cores
    nblk = -(-rpc // P)
    last_rows = rpc - (nblk - 1) * P

    prep_lp = _prep_op(inputs["lp_rows"], inputs["lp_cols"], inputs["lp_vals"],
                       n_cores, rpc, nblk, half)
    prep_hp = _prep_op(inputs["hp_rows"], inputs["hp_cols"], inputs["hp_vals"],
                       n_cores, rpc, nblk, half)
    idx_a, vals_a, rblk_a, segs, tc, ct0, tt = _pack(
        n_cores, nblk, prep_lp, prep_hp)

    nc = _build(n, d, n_cores, rpc, nblk, last_rows, segs, tc, ct0, tt, half)

    x0b = x[:half].astype(ml_dtypes.bfloat16)
    x1b = x[half:].astype(ml_dtypes.bfloat16)
    wt = np.ascontiguousarray(np.asarray(inputs["W"], np.float32).T)
    bvec = np.asarray(inputs["b"], np.float32).reshape(1, d)
    aw = np.asarray(inputs["alpha_w"], np.float32).reshape(1, d)
    ab = np.asarray(inputs["alpha_b"], np.float32).reshape(1, 1)
    in_maps = []
    for c in range(n_cores):
        in_maps.append({
            "x0b": x0b, "x1b": x1b,
            "x_own": np.ascontiguousarray(x[c * rpc : (c + 1) * rpc]),
            "wt": wt, "bvec": bvec, "aw": aw, "ab": ab,
            "idx": np.ascontiguousarray(idx_a[c]),
            "vals": np.ascontiguousarray(vals_a[c]),
            "rblk": np.ascontiguousarray(rblk_a[c]),
        })
    return nc, in_maps, rpc


def _run(inputs, n_cores=8, trace=False):
    from concourse.bass_utils import run_bass_kernel_spmd

    nc, in_maps, rpc = _make(inputs, n_cores)
    res = run_bass_kernel_spmd(nc, in_maps, core_ids=list(range(n_cores)),
                               trace=trace)
    out = np.concatenate([res.results[c]["out"] for c in range(n_cores)], 0)
    alpha = np.concatenate([res.results[c]["alpha"] for c in range(n_cores)], 0)
    return (out, alpha), res


def kernel(**inputs):
    (out, alpha), _ = _run(inputs, trace=False)
    return out, alpha
